# revision 4
# baseline (speedup 1.0000x reference)
"""JointAtt (dense_cnn) Trainium2 Bass kernel — v3 (engine-balanced fp16).

Per slice (n, g): x slice [128, 4096] fp16 in SBUF.
  PE:     Yh/Yw pooling+conv1 fused, J=16 octaves -> 4 matmuls of N=1024
          each direction (PSUM f32 accumulate); 2 small attention matmuls.
  GpSimd: pairwise-add trees reduce Yh/Yw [16,64,16] -> Y [16,128].
  Scalar: T = relu(Y + b1eff + 3); sigmoid for a_w [128,64]; sigmoid with
          broadcast AP materializes AH_exp [128, 64x64] fp16 directly from
          PSUM (so both DVE multiplies get unit-stride fp16 operands).
  DVE:    T3 = max(Y + b1eff, -3) (tensor_scalar);
          HS = min(T,6)*T3 (scalar_tensor_tensor, fp16);
          OUT = X * AH_exp * aw_b  (two 2x-rate fp16 tensor_tensor).
  DMA:    1 contiguous 1 MB load (sync ring) + 1 MB store (scalar ring)
          per slice; channel shuffle + fp32 conversion on the host.
"""

import numpy as np

import concourse.bass as bass
import concourse.bacc as bacc
import concourse.mybir as mybir
import concourse.tile as tile
from concourse.bass_utils import run_bass_kernel_spmd

F32 = mybir.dt.float32
F16 = mybir.dt.float16

N_CORES = 8
NB = 2          # batches per core
C = 512
G = 4           # groups
CG = 128        # channels per group
H = 64
W = 64
HW = H * W
S = NB * G      # slices per core
MIP = 16        # conv1 output channels
J = 8           # pooling octave width (one 2KB PSUM bank: N=512 fp32)
EPS = 1e-5

_NC_CACHE = None


def _build_bass():
    nc = bacc.Bacc(None, target_bir_lowering=False)

    x_d = nc.dram_tensor("x", [S, CG, HW], F16, kind="ExternalInput")
    w1t_d = nc.dram_tensor("w1t", [CG, MIP], F16, kind="ExternalInput")
    whw_d = nc.dram_tensor("whw", [MIP, 2 * CG], F16, kind="ExternalInput")
    bact_d = nc.dram_tensor("bact", [MIP, 1], F32, kind="ExternalInput")
    b1f_d = nc.dram_tensor("b1f", [MIP, 1], F32, kind="ExternalInput")
    bhw_d = nc.dram_tensor("bhw", [CG, 2], F32, kind="ExternalInput")
    out_d = nc.dram_tensor("out", [S, CG, HW], F16, kind="ExternalOutput")

    Relu = mybir.ActivationFunctionType.Relu
    Sigmoid = mybir.ActivationFunctionType.Sigmoid
    ADD = mybir.AluOpType.add
    MAX = mybir.AluOpType.max
    MIN = mybir.AluOpType.min
    MULT = mybir.AluOpType.mult

    with tile.TileContext(nc) as tc:
        with (
            tc.tile_pool(name="consts", bufs=1) as consts,
            tc.tile_pool(name="xp", bufs=3) as xp,
            tc.tile_pool(name="op", bufs=3) as op,
            tc.tile_pool(name="ahp", bufs=2) as ahp,
            tc.tile_pool(name="ps", bufs=1, space="PSUM") as ps,
            tc.tile_pool(name="sm", bufs=3) as sm,
        ):
            w1t = consts.tile([CG, MIP], F16)
            nc.sync.dma_start(out=w1t, in_=w1t_d[:])
            whw = consts.tile([MIP, 2 * CG], F16)
            nc.sync.dma_start(out=whw, in_=whw_d[:])
            bact = consts.tile([MIP, 1], F32)
            nc.sync.dma_start(out=bact, in_=bact_d[:])
            b1f = consts.tile([MIP, 1], F32)
            nc.sync.dma_start(out=b1f, in_=b1f_d[:])
            bhw = consts.tile([CG, 2], F32)
            nc.sync.dma_start(out=bhw, in_=bhw_d[:])
            wht = whw[:, 0:CG]
            wwt = whw[:, CG:]
            bh = bhw[:, 0:1]
            bw = bhw[:, 1:2]

            for s in range(S):
                # ---- load x slice: one fully-contiguous 1 MB DMA.
                X = xp.tile([CG, HW], F16, name="X")
                nc.sync.dma_start(out=X, in_=x_d[s])
                Xr = X.rearrange("p (h w) -> p h w", h=H)

                # ---- pooling sums fused with conv1 on the TensorEngine
                Yh = ps.tile([MIP, H, J], F32, name="Yh", tag="Yh", bufs=2)
                for k in range(HW // (H * J)):
                    nc.tensor.matmul(
                        Yh,
                        w1t,
                        Xr[:, :, J * k : J * (k + 1)],
                        start=(k == 0),
                        stop=(k == HW // (H * J) - 1),
                    )
                Yw = ps.tile([MIP, W, J], F32, name="Yw", tag="Yw", bufs=2)
                for k in range(HW // (W * J)):
                    nc.tensor.matmul(
                        Yw,
                        w1t,
                        Xr[:, J * k : J * (k + 1), :].transpose([0, 2, 1]),
                        start=(k == 0),
                        stop=(k == HW // (W * J) - 1),
                    )

                # ---- finish the reduction.  Yh: DVE tensor_reduce straight
                # out of PSUM (only one PSUM operand allowed per DVE op).
                # Yw: Scalar evacuates PSUM->SBUF, GpSimd runs the pairwise
                # add tree (GpSimd cannot read PSUM).
                Y = sm.tile([MIP, H + W], F32, name="Y", tag="Y")
                nc.vector.tensor_reduce(
                    out=Y[:, 0:H], in_=Yh, axis=mybir.AxisListType.X, op=ADD
                )
                YwS = sm.tile([MIP, W, J], F32, name="YwS", tag="YwS", bufs=2)
                nc.scalar.activation(
                    out=YwS, in_=Yw, func=mybir.ActivationFunctionType.Copy
                )
                cur = YwS
                width = J
                while width > 2:
                    nxt = sm.tile(
                        [MIP, W, width // 2], F32,
                        name=f"Rw{width//2}", tag=f"Rw{width//2}", bufs=2,
                    )
                    nc.gpsimd.tensor_tensor(
                        out=nxt, in0=cur[:, :, 0:width:2],
                        in1=cur[:, :, 1:width:2], op=ADD,
                    )
                    cur = nxt
                    width //= 2
                Yw_f = Y[:, H:].rearrange("p (w one) -> p w one", one=1)
                nc.gpsimd.tensor_tensor(
                    out=Yw_f, in0=cur[:, :, 0:1], in1=cur[:, :, 1:2], op=ADD
                )

                # ---- hswish: HS = min(relu(Y+b+3), 6) * max(Y+b, -3)
                T = sm.tile([MIP, H + W], F32, name="T", tag="T")
                nc.scalar.activation(out=T, in_=Y, func=Relu, bias=bact, scale=1.0)
                T3 = sm.tile([MIP, H + W], F32, name="T3", tag="T3")
                nc.vector.tensor_scalar(
                    out=T3, in0=Y, scalar1=b1f, scalar2=-3.0, op0=ADD, op1=MAX
                )
                HS = sm.tile([MIP, H + W], F16, name="HS", tag="HS")
                nc.vector.scalar_tensor_tensor(
                    out=HS, in0=T, scalar=6.0, in1=T3, op0=MIN, op1=MULT
                )

                # ---- attention logits (K=16 matmuls)
                APs = ps.tile([CG, H + W], F32, name="APs", tag="APs", bufs=2)
                nc.tensor.matmul(APs[:, 0:H], wht, HS[:, 0:H], start=True, stop=True)
                nc.tensor.matmul(APs[:, H:], wwt, HS[:, H:], start=True, stop=True)

                # ---- sigmoid; a_h materialized broadcast so the big DVE
                # multiplies both run in the 2x fp16 mode.
                AHE = ahp.tile([CG, H, W], F16, name="AHE")
                nc.scalar.activation(
                    out=AHE,
                    in_=APs[:, 0:H].unsqueeze(2).broadcast_to([CG, H, W]),
                    func=Sigmoid,
                    bias=bh,
                )
                AW = sm.tile([CG, W], F16, name="AW", tag="AW")
                nc.scalar.activation(out=AW, in_=APs[:, H:], func=Sigmoid, bias=bw)

                # ---- out = x * a_h_exp * a_w  (both 2x-rate on DVE)
                OUT = op.tile([CG, HW], F16, name="OUT")
                OUTr = OUT.rearrange("p (h w) -> p h w", h=H)
                aw_b = AW.unsqueeze(1).broadcast_to([CG, H, W])
                nc.vector.tensor_tensor(out=OUTr, in0=Xr, in1=AHE, op=MULT)
                nc.vector.tensor_tensor(out=OUTr, in0=OUTr, in1=aw_b, op=MULT)

                # ---- store: one contiguous 1 MB DMA on the scalar ring.
                nc.scalar.dma_start(out=out_d[s], in_=OUT)

    nc.finalize()
    return nc


def _get_nc():
    global _NC_CACHE
    if _NC_CACHE is None:
        _NC_CACHE = _build_bass()
    return _NC_CACHE


def _prep_weights(W1, b1, gamma, beta, mean, var, Wh, bh, Ww, bw):
    W1 = np.asarray(W1, np.float64)
    b1 = np.asarray(b1, np.float64)
    gamma = np.asarray(gamma, np.float64)
    beta = np.asarray(beta, np.float64)
    mean = np.asarray(mean, np.float64)
    var = np.asarray(var, np.float64)
    Wh = np.asarray(Wh, np.float64)
    Ww = np.asarray(Ww, np.float64)
    bh = np.asarray(bh, np.float64)
    bw = np.asarray(bw, np.float64)

    scale = gamma / np.sqrt(var + EPS)                    # (MIP,)
    w1eff = (W1 * scale[:, None]) / float(W)              # (MIP, CG); mean 1/64
    b1eff = scale * (b1 - mean) + beta                    # (MIP,)
    bact = (b1eff + 3.0).astype(np.float32)[:, None]      # (MIP, 1)
    b1f = b1eff.astype(np.float32)[:, None]               # (MIP, 1)

    w1t = np.ascontiguousarray(w1eff.T.astype(np.float16))            # (CG, MIP)
    whw = np.concatenate([(Wh / 6.0).T, (Ww / 6.0).T], axis=1)        # (MIP, 2CG)
    whw = np.ascontiguousarray(whw.astype(np.float16))
    bhw = np.ascontiguousarray(
        np.stack([bh, bw], axis=1).astype(np.float32)
    )                                                     # (CG, 2)
    return w1t, whw, bact, b1f, bhw


def run(inputs: dict, trace: bool = False):
    """Run on 8 NeuronCores. Returns (out [16,512,64,64] fp32, results)."""
    x = np.asarray(inputs["x"], dtype=np.float32)
    n = x.shape[0]
    assert x.shape == (n, C, H, W) and n == N_CORES * NB, x.shape

    w1t, whw, bact, b1f, bhw = _prep_weights(
        inputs["W1"], inputs["b1"], inputs["gamma"], inputs["beta"],
        inputs["mean"], inputs["var"], inputs["Wh"], inputs["bh"],
        inputs["Ww"], inputs["bw"],
    )

    # fp16, pre-sliced per core: [core, slice(b,g), 128, 4096]
    x16 = np.ascontiguousarray(
        x.astype(np.float16).reshape(N_CORES, S, CG, HW)
    )

    nc = _get_nc()
    core_ids = list(range(N_CORES))
    in_maps = []
    for k in core_ids:
        in_maps.append(
            {
                "x": x16[k],
                "w1t": w1t,
                "whw": whw,
                "bact": bact,
                "b1f": b1f,
                "bhw": bhw,
            }
        )

    res = run_bass_kernel_spmd(nc, in_maps, core_ids, trace=trace)
    out16 = np.stack([res.results[k]["out"] for k in core_ids])  # (8,8,128,HW)
    # group-major == natural channel order; then apply the channel shuffle
    # c' = (c % 4) * 128 + c // 4 on the host, with the fp16->fp32 upcast.
    nat = out16.astype(np.float32).reshape(n, C, H, W)
    out = np.ascontiguousarray(
        nat.reshape(n, CG, G, H, W).transpose(0, 2, 1, 3, 4).reshape(n, C, H, W)
    )
    return out, res


def kernel(**inputs) -> np.ndarray:
    out, _ = run(inputs, trace=False)
    return out


def exec_time_ns(res):
    return res.exec_time_ns


# revision 5
# speedup vs baseline: 1.1401x; 1.1401x over previous
"""JointAtt (dense_cnn) Trainium2 Bass kernel — v3 (engine-balanced fp16).

Per slice (n, g): x slice [128, 4096] fp16 in SBUF.
  PE:     Yh/Yw pooling+conv1 fused, J=16 octaves -> 4 matmuls of N=1024
          each direction (PSUM f32 accumulate); 2 small attention matmuls.
  GpSimd: pairwise-add trees reduce Yh/Yw [16,64,16] -> Y [16,128].
  Scalar: T = relu(Y + b1eff + 3); sigmoid for a_w [128,64]; sigmoid with
          broadcast AP materializes AH_exp [128, 64x64] fp16 directly from
          PSUM (so both DVE multiplies get unit-stride fp16 operands).
  DVE:    T3 = max(Y + b1eff, -3) (tensor_scalar);
          HS = min(T,6)*T3 (scalar_tensor_tensor, fp16);
          OUT = X * AH_exp * aw_b  (two 2x-rate fp16 tensor_tensor).
  DMA:    1 contiguous 1 MB load (sync ring) + 1 MB store (scalar ring)
          per slice; channel shuffle + fp32 conversion on the host.
"""

import numpy as np

import concourse.bass as bass
import concourse.bacc as bacc
import concourse.mybir as mybir
import concourse.tile as tile
from concourse.bass_utils import run_bass_kernel_spmd

F32 = mybir.dt.float32
F16 = mybir.dt.float16

N_CORES = 8
NB = 2          # batches per core
C = 512
G = 4           # groups
CG = 128        # channels per group
H = 64
W = 64
HW = H * W
S = NB * G      # slices per core
MIP = 16        # conv1 output channels
J = 8           # pooling octave width (one 2KB PSUM bank: N=512 fp32)
EPS = 1e-5

_NC_CACHE = None


def _build_bass():
    nc = bacc.Bacc(None, target_bir_lowering=False)

    x_d = nc.dram_tensor("x", [S, CG, HW], F16, kind="ExternalInput")
    w1t_d = nc.dram_tensor("w1t", [CG, MIP], F16, kind="ExternalInput")
    whw_d = nc.dram_tensor("whw", [MIP, 2 * CG], F16, kind="ExternalInput")
    bact_d = nc.dram_tensor("bact", [MIP, 1], F32, kind="ExternalInput")
    b1f_d = nc.dram_tensor("b1f", [MIP, 1], F32, kind="ExternalInput")
    bhw_d = nc.dram_tensor("bhw", [CG, 2], F32, kind="ExternalInput")
    out_d = nc.dram_tensor("out", [S, CG, HW], F16, kind="ExternalOutput")

    Relu = mybir.ActivationFunctionType.Relu
    Sigmoid = mybir.ActivationFunctionType.Sigmoid
    ADD = mybir.AluOpType.add
    MAX = mybir.AluOpType.max
    MIN = mybir.AluOpType.min
    MULT = mybir.AluOpType.mult

    with tile.TileContext(nc) as tc:
        with (
            tc.tile_pool(name="consts", bufs=1) as consts,
            tc.tile_pool(name="xp", bufs=4) as xp,
            tc.tile_pool(name="op", bufs=3) as op,
            tc.tile_pool(name="ahp", bufs=3) as ahp,
            tc.tile_pool(name="ps", bufs=1, space="PSUM") as ps,
            tc.tile_pool(name="sm", bufs=3) as sm,
        ):
            w1t = consts.tile([CG, MIP], F16)
            nc.scalar.dma_start(out=w1t, in_=w1t_d[:])
            whw = consts.tile([MIP, 2 * CG], F16)
            nc.scalar.dma_start(out=whw, in_=whw_d[:])
            bact = consts.tile([MIP, 1], F32)
            nc.scalar.dma_start(out=bact, in_=bact_d[:])
            b1f = consts.tile([MIP, 1], F32)
            nc.scalar.dma_start(out=b1f, in_=b1f_d[:])
            bhw = consts.tile([CG, 2], F32)
            nc.scalar.dma_start(out=bhw, in_=bhw_d[:])
            wht = whw[:, 0:CG]
            wwt = whw[:, CG:]
            bh = bhw[:, 0:1]
            bw = bhw[:, 1:2]

            for s in range(S):
                # ---- load x slice: one fully-contiguous 1 MB DMA.
                X = xp.tile([CG, HW], F16, name="X")
                nc.sync.dma_start(out=X, in_=x_d[s])
                Xr = X.rearrange("p (h w) -> p h w", h=H)

                # ---- pooling sums fused with conv1 on the TensorEngine
                Yh = ps.tile([MIP, H, J], F32, name="Yh", tag="Yh", bufs=2)
                for k in range(HW // (H * J)):
                    nc.tensor.matmul(
                        Yh,
                        w1t,
                        Xr[:, :, J * k : J * (k + 1)],
                        start=(k == 0),
                        stop=(k == HW // (H * J) - 1),
                    )
                # Yw pass: fully-contiguous flat 512-column blocks; block k
                # covers h in [8k, 8k+8), accumulating into buckets (h%8, w).
                Yw = ps.tile([MIP, J, W], F32, name="Yw", tag="Yw", bufs=2)
                for k in range(HW // (W * J)):
                    nc.tensor.matmul(
                        Yw,
                        w1t,
                        X[:, J * W * k : J * W * (k + 1)],
                        start=(k == 0),
                        stop=(k == HW // (W * J) - 1),
                    )

                # ---- finish the reduction.  Yh: DVE tensor_reduce straight
                # out of PSUM (only one PSUM operand allowed per DVE op).
                # Yw: Scalar evacuates PSUM->SBUF, GpSimd runs the pairwise
                # add tree (GpSimd cannot read PSUM).
                Y = sm.tile([MIP, H + W], F32, name="Y", tag="Y")
                nc.vector.tensor_reduce(
                    out=Y[:, 0:H], in_=Yh, axis=mybir.AxisListType.X, op=ADD
                )
                YwS = sm.tile([MIP, J, W], F32, name="YwS", tag="YwS", bufs=2)
                nc.scalar.activation(
                    out=YwS, in_=Yw, func=mybir.ActivationFunctionType.Copy
                )
                cur = YwS
                width = J
                while width > 2:
                    nxt = sm.tile(
                        [MIP, width // 2, W], F32,
                        name=f"Rw{width//2}", tag=f"Rw{width//2}", bufs=2,
                    )
                    nc.gpsimd.tensor_tensor(
                        out=nxt, in0=cur[:, 0:width:2, :],
                        in1=cur[:, 1:width:2, :], op=ADD,
                    )
                    cur = nxt
                    width //= 2
                Yw_f = Y[:, H:].rearrange("p (one w) -> p one w", one=1)
                nc.gpsimd.tensor_tensor(
                    out=Yw_f, in0=cur[:, 0:1, :], in1=cur[:, 1:2, :], op=ADD
                )

                # ---- hswish: HS = min(relu(Y+b+3), 6) * max(Y+b, -3)
                T = sm.tile([MIP, H + W], F32, name="T", tag="T")
                nc.scalar.activation(out=T, in_=Y, func=Relu, bias=bact, scale=1.0)
                T3 = sm.tile([MIP, H + W], F32, name="T3", tag="T3")
                nc.vector.tensor_scalar(
                    out=T3, in0=Y, scalar1=b1f, scalar2=-3.0, op0=ADD, op1=MAX
                )
                HS = sm.tile([MIP, H + W], F16, name="HS", tag="HS")
                nc.vector.scalar_tensor_tensor(
                    out=HS, in0=T, scalar=6.0, in1=T3, op0=MIN, op1=MULT
                )

                # ---- attention logits (K=16 matmuls)
                APs = ps.tile([CG, H + W], F32, name="APs", tag="APs", bufs=4)
                nc.tensor.matmul(APs[:, 0:H], wht, HS[:, 0:H], start=True, stop=True)
                nc.tensor.matmul(APs[:, H:], wwt, HS[:, H:], start=True, stop=True)

                # ---- sigmoid; a_h materialized broadcast so the big DVE
                # multiplies both run in the 2x fp16 mode.
                AHE = ahp.tile([CG, H, W], F16, name="AHE")
                nc.scalar.activation(
                    out=AHE,
                    in_=APs[:, 0:H].unsqueeze(2).broadcast_to([CG, H, W]),
                    func=Sigmoid,
                    bias=bh,
                )
                AW = sm.tile([CG, W], F16, name="AW", tag="AW")
                nc.scalar.activation(out=AW, in_=APs[:, H:], func=Sigmoid, bias=bw)

                # ---- out = x * a_h_exp * a_w  (both 2x-rate on DVE)
                OUT = op.tile([CG, HW], F16, name="OUT")
                OUTr = OUT.rearrange("p (h w) -> p h w", h=H)
                aw_b = AW.unsqueeze(1).broadcast_to([CG, H, W])
                nc.vector.tensor_tensor(out=OUTr, in0=Xr, in1=AHE, op=MULT)
                nc.vector.tensor_tensor(out=OUTr, in0=OUTr, in1=aw_b, op=MULT)

                # ---- store: one contiguous 1 MB DMA on the scalar ring.
                nc.scalar.dma_start(out=out_d[s], in_=OUT)

    nc.finalize()
    return nc


def _get_nc():
    global _NC_CACHE
    if _NC_CACHE is None:
        _NC_CACHE = _build_bass()
    return _NC_CACHE


def _prep_weights(W1, b1, gamma, beta, mean, var, Wh, bh, Ww, bw):
    W1 = np.asarray(W1, np.float64)
    b1 = np.asarray(b1, np.float64)
    gamma = np.asarray(gamma, np.float64)
    beta = np.asarray(beta, np.float64)
    mean = np.asarray(mean, np.float64)
    var = np.asarray(var, np.float64)
    Wh = np.asarray(Wh, np.float64)
    Ww = np.asarray(Ww, np.float64)
    bh = np.asarray(bh, np.float64)
    bw = np.asarray(bw, np.float64)

    scale = gamma / np.sqrt(var + EPS)                    # (MIP,)
    w1eff = (W1 * scale[:, None]) / float(W)              # (MIP, CG); mean 1/64
    b1eff = scale * (b1 - mean) + beta                    # (MIP,)
    bact = (b1eff + 3.0).astype(np.float32)[:, None]      # (MIP, 1)
    b1f = b1eff.astype(np.float32)[:, None]               # (MIP, 1)

    w1t = np.ascontiguousarray(w1eff.T.astype(np.float16))            # (CG, MIP)
    whw = np.concatenate([(Wh / 6.0).T, (Ww / 6.0).T], axis=1)        # (MIP, 2CG)
    whw = np.ascontiguousarray(whw.astype(np.float16))
    bhw = np.ascontiguousarray(
        np.stack([bh, bw], axis=1).astype(np.float32)
    )                                                     # (CG, 2)
    return w1t, whw, bact, b1f, bhw


def run(inputs: dict, trace: bool = False):
    """Run on 8 NeuronCores. Returns (out [16,512,64,64] fp32, results)."""
    x = np.asarray(inputs["x"], dtype=np.float32)
    n = x.shape[0]
    assert x.shape == (n, C, H, W) and n == N_CORES * NB, x.shape

    w1t, whw, bact, b1f, bhw = _prep_weights(
        inputs["W1"], inputs["b1"], inputs["gamma"], inputs["beta"],
        inputs["mean"], inputs["var"], inputs["Wh"], inputs["bh"],
        inputs["Ww"], inputs["bw"],
    )

    # fp16, pre-sliced per core: [core, slice(b,g), 128, 4096]
    x16 = np.ascontiguousarray(
        x.astype(np.float16).reshape(N_CORES, S, CG, HW)
    )

    nc = _get_nc()
    core_ids = list(range(N_CORES))
    in_maps = []
    for k in core_ids:
        in_maps.append(
            {
                "x": x16[k],
                "w1t": w1t,
                "whw": whw,
                "bact": bact,
                "b1f": b1f,
                "bhw": bhw,
            }
        )

    res = run_bass_kernel_spmd(nc, in_maps, core_ids, trace=trace)
    out16 = np.stack([res.results[k]["out"] for k in core_ids])  # (8,8,128,HW)
    # group-major == natural channel order; then apply the channel shuffle
    # c' = (c % 4) * 128 + c // 4 on the host, with the fp16->fp32 upcast.
    nat = out16.astype(np.float32).reshape(n, C, H, W)
    out = np.ascontiguousarray(
        nat.reshape(n, CG, G, H, W).transpose(0, 2, 1, 3, 4).reshape(n, C, H, W)
    )
    return out, res


def kernel(**inputs) -> np.ndarray:
    out, _ = run(inputs, trace=False)
    return out


def exec_time_ns(res):
    return res.exec_time_ns
